# revision 13
# baseline (speedup 1.0000x reference)
"""Trainium2 Bass kernel for the ExpertVectorSystem MoE-routing problem.

Reference computation (all fp32):
    we = expert_weights @ expert_vectors              # [B, D]
    for each layer i (8 layers, rank r_i):
        h_i   = relu(we @ w1_i + b1_i)                # [B, 2r]
        out_i = tanh(h_i @ w2_i + b2_i) * 0.1         # [B, r]
    out = concat(out_i, axis=-1)                      # [B, sum(r)]

Strategy: data-parallel over the batch across 8 NeuronCores (2048 rows
each); the tiny expert_vectors / per-layer MLP weights are replicated.

All matmuls run in float32r (fp32 bits, reduced-precision PE mode): at
moving-dim >= 256 it streams 1 row/cycle like bf16 (4x faster than
strict fp32) while measuring ~16x more accurate than bf16.

Per-core device program (fp32r matmuls, fp32 PSUM accumulation):
  phase 0: weT[d, b] = (expert_vectors^T @ expert_weights^T) computed on
           PE from a host-pre-transposed ewT shard; a ones-row is
           appended (weT_ext, K=65) so b1 folds into an augmented w1
           (homogeneous coordinates).
  per layer, per batch-group of 512 columns:
    stage 1: hT chunks [128, 512] = w1_aug_chunk^T @ weT_ext  (K=65,
             N=512), relu on ScalarE into SBUF.
    stage 2: out tiles [128(batch), r-chunk in [256,512]] accumulated
             over the 2r/128 K-chunks in PSUM (lhsT = hT chunk slices),
             then tanh on ScalarE, *0.1 on VectorE, DMA to the output
             column slice.

b1 is always folded (free: K=64 -> 65).  If any b2 is nonzero, the same
homogeneous trick adds one extra K-chunk whose first h row is constant 1
and whose w2 rows carry b2 (zero for the given problem, so normally off).
"""

import contextlib
import ctypes
import os
import sys
import types

import numpy as np

import concourse.bass as bass
import concourse.mybir as mybir
import concourse.tile as tile
from concourse.bass_utils import run_bass_kernel_spmd

B = 16384
E = 16
D = 64
RANKS = [256, 384, 512, 640, 768, 896, 1024, 1152]
STRENGTH = 0.1
NCORES = 8
BL = B // NCORES          # 2048 rows per core
GCOLS = 512               # batch columns per stage-1 group
NGROUPS = BL // GCOLS     # 4
NTILES_PER_GROUP = GCOLS // 128  # 4

F32R = mybir.dt.float32r
F32 = mybir.dt.float32

OUT_COLS = sum(RANKS)     # 5888


def _split_excess_waits(nc):
    """Rewrite instructions carrying >1 sync wait.

    The walrus build in this container accepts at most ONE sync wait per
    instruction ("Too many sync wait commands", CoreV*GenImpl
    setupSyncWait), while Tile's wait assignment freely attaches several.
    Hoist the extra waits onto standalone InstEventSemaphore instructions
    (what BassEngine.wait_ge emits) inserted immediately before the
    instruction on the same engine — same-engine program order makes this
    semantically identical.
    """
    n_split = 0
    for f in nc.m.functions:
        for bb in f.blocks:
            out = []
            dirty = False
            for ins in bb.instructions:
                si = ins.sync_info
                waits = list(si.on_wait) if si is not None else []
                if len(waits) > 1:
                    dirty = True
                    for k, w in enumerate(waits[:-1]):
                        out.append(
                            mybir.InstEventSemaphore(
                                name=f"{ins.name}_xw{k}",
                                engine=ins.engine,
                                ins=[],
                                outs=[],
                                sync_info=mybir.SyncInfo(
                                    on_wait=[w], on_update=[]
                                ),
                            )
                        )
                        n_split += 1
                    ins.sync_info = mybir.SyncInfo(
                        on_wait=[waits[-1]], on_update=list(si.on_update)
                    )
                out.append(ins)
            if dirty:
                bb.instructions = out
    return n_split


def _rchunks(r):
    """Split a layer's output width r into nearly-even chunks <= 512.

    Every chunk ends up in [256, 512] for the given ranks, which keeps
    float32r matmuls at the full 1-row/cycle rate.
    """
    n = -(-r // 512)
    sizes = []
    rem = r
    for i in range(n):
        s = -(-rem // (n - i))
        sizes.append(s)
        rem -= s
    offs = [0]
    for s in sizes[:-1]:
        offs.append(offs[-1] + s)
    return list(zip(offs, sizes))


def _build_program(with_b2: bool):
    """Build the per-core Bass program (identical on every core)."""
    kcs = [2 * r // 128 + (1 if with_b2 else 0) for r in RANKS]
    w1_cols = [kc * 128 for kc in kcs]           # per-layer w1_aug col count
    W1TOT = sum(w1_cols)

    nc = bass.Bass()
    # ewT carries an appended ones-row; v_aug is block-diagonal so the
    # phase-0 matmul emits weT_ext = [[we^T], [ones]] directly (no memset:
    # this walrus rejects Memset on float32r).
    ewT_d = nc.declare_dram_parameter("ewT", [E + 1, BL], F32R, isOutput=False)
    v_d = nc.declare_dram_parameter("v", [E + 1, D + 1], F32R, isOutput=False)
    w1_d = nc.declare_dram_parameter("w1cat", [D + 1, W1TOT], F32R, isOutput=False)
    w2_d = [
        nc.declare_dram_parameter(f"w2_{i}", [128, kcs[i] * RANKS[i]], F32R,
                                  isOutput=False)
        for i in range(len(RANKS))
    ]
    out_d = nc.declare_dram_parameter("out", [BL, OUT_COLS], F32, isOutput=True)

    with tile.TileContext(nc) as tc:
        with (
            tc.tile_pool(name="const", bufs=1) as cpool,
            tc.tile_pool(name="hpsum", bufs=4, space="PSUM") as hpsum,
            tc.tile_pool(name="opsum", bufs=4, space="PSUM") as opsum,
        ):
            # ---- phase 0: load constants, compute weT_ext [65, BL] ----
            v_sb = cpool.tile([E + 1, D + 1], F32R, name="v_sb")
            nc.sync.dma_start(v_sb[:], v_d[:])
            w1_sb = cpool.tile([D + 1, W1TOT], F32R, name="w1_sb")
            for i in range(len(RANKS)):
                off = sum(w1_cols[:i])
                nc.sync.dma_start(
                    w1_sb[:, off:off + w1_cols[i]], w1_d[:, off:off + w1_cols[i]]
                )
            weT = cpool.tile([D + 1, BL], F32R, name="weT")

            with tc.tile_pool(name="ew", bufs=1) as ewpool:
                ewT_sb = ewpool.tile([E + 1, BL], F32R, name="ewT_sb")
                nc.sync.dma_start(ewT_sb[:], ewT_d[:])
                for g in range(NGROUPS):
                    wp = hpsum.tile([D + 1, GCOLS], F32, tag="hp", name="wp")
                    nc.tensor.matmul(
                        wp[:], v_sb[:], ewT_sb[:, g * GCOLS:(g + 1) * GCOLS],
                        start=True, stop=True,
                    )
                    nc.vector.tensor_copy(
                        weT[0:D + 1, g * GCOLS:(g + 1) * GCOLS], wp[:]
                    )

            # ---- main: layer-outer, batch-group inner ----
            with (
                tc.tile_pool(name="w2", bufs=1) as w2pool,
                tc.tile_pool(name="h", bufs=1) as hpool,
                tc.tile_pool(name="osb", bufs=6) as osb,
            ):
                col_off = 0
                for li, r in enumerate(RANKS):
                    kc = kcs[li]
                    w1_off = sum(w1_cols[:li])
                    rch = _rchunks(r)

                    w2_sb = []
                    for c in range(kc):
                        t = w2pool.tile([128, r], F32R, tag=f"w2_{c}",
                                        name=f"w2_{li}_{c}")
                        nc.sync.dma_start(t[:], w2_d[li][:, c * r:(c + 1) * r])
                        w2_sb.append(t)

                    for g in range(NGROUPS):
                        h_sb = []
                        for c in range(kc):
                            hp = hpsum.tile([128, GCOLS], F32, tag="hp",
                                            name=f"hp_{li}_{g}_{c}")
                            nc.tensor.matmul(
                                hp[:],
                                w1_sb[:, w1_off + c * 128: w1_off + (c + 1) * 128],
                                weT[:, g * GCOLS:(g + 1) * GCOLS],
                                start=True, stop=True,
                            )
                            ht = hpool.tile([128, GCOLS], F32R, tag=f"h_{c}",
                                            name=f"h_{li}_{g}_{c}")
                            nc.scalar.activation(
                                ht[:], hp[:], mybir.ActivationFunctionType.Relu
                            )
                            h_sb.append(ht)

                        for j in range(NTILES_PER_GROUP):
                            row0 = g * GCOLS + j * 128
                            for (rc_off, rc_sz) in rch:
                                op = opsum.tile([128, rc_sz], F32, tag="op",
                                                name=f"op_{li}_{g}_{j}_{rc_off}")
                                for c in range(kc):
                                    nc.tensor.matmul(
                                        op[:],
                                        h_sb[c][:, j * 128:(j + 1) * 128],
                                        w2_sb[c][:, rc_off:rc_off + rc_sz],
                                        start=(c == 0), stop=(c == kc - 1),
                                    )
                                ot = osb.tile([128, rc_sz], F32, tag="ot",
                                              name=f"ot_{li}_{g}_{j}_{rc_off}")
                                nc.scalar.activation(
                                    ot[:], op[:], mybir.ActivationFunctionType.Tanh
                                )
                                nc.vector.tensor_scalar_mul(ot[:], ot[:], STRENGTH)
                                nc.sync.dma_start(
                                    out_d[row0:row0 + 128,
                                          col_off + rc_off:col_off + rc_off + rc_sz],
                                    ot[:],
                                )
                    col_off += r
    _split_excess_waits(nc)
    return nc


_CACHE = {}


def _get_program(with_b2: bool):
    if with_b2 not in _CACHE:
        _CACHE[with_b2] = _build_program(with_b2)
    return _CACHE[with_b2]


def _prepare_inputs(inputs, with_b2):
    """Host-side: transpose/augment and shard per core (all fp32 bits)."""
    ew = np.asarray(inputs["expert_weights"], dtype=np.float32)
    v = np.asarray(inputs["expert_vectors"], dtype=np.float32)

    # [E+1, B]: last row is all-ones (drives weT_ext's homogeneous row)
    ewT = np.concatenate([ew.T, np.ones((1, B), np.float32)], axis=0)
    # [E+1, D+1] block-diagonal: top-left = v, bottom-right = 1
    v_aug = np.zeros((E + 1, D + 1), np.float32)
    v_aug[:E, :D] = v
    v_aug[E, D] = 1.0

    w1_parts = []
    w2_parts = []
    for i, r in enumerate(RANKS):
        w1 = np.asarray(inputs[f"w1_{i}"], dtype=np.float32)   # [D, 2r]
        b1 = np.asarray(inputs[f"b1_{i}"], dtype=np.float32)   # [2r]
        w2 = np.asarray(inputs[f"w2_{i}"], dtype=np.float32)   # [2r, r]
        b2 = np.asarray(inputs[f"b2_{i}"], dtype=np.float32)   # [r]

        w1_aug = np.concatenate([w1, b1[None, :]], axis=0)     # [D+1, 2r]
        if with_b2:
            # extra 128 h-columns: first is the constant-1 unit
            # (weight col 0, b1 entry 1), rest identically zero.
            pad = np.zeros((D + 1, 128), np.float32)
            pad[D, 0] = 1.0
            w1_aug = np.concatenate([w1_aug, pad], axis=1)     # [D+1, 2r+128]
            w2pad = np.zeros((128, r), np.float32)
            w2pad[0, :] = b2
            w2 = np.concatenate([w2, w2pad], axis=0)           # [2r+128, r]
        kc = w2.shape[0] // 128
        w2_k = np.ascontiguousarray(
            w2.reshape(kc, 128, r).transpose(1, 0, 2).reshape(128, kc * r)
        )
        w1_parts.append(w1_aug)
        w2_parts.append(w2_k)
    w1cat = np.ascontiguousarray(np.concatenate(w1_parts, axis=1))

    in_maps = []
    for core in range(NCORES):
        m = {
            "ewT": np.ascontiguousarray(ewT[:, core * BL:(core + 1) * BL]),
            "v": v_aug,
            "w1cat": w1cat,
        }
        for i in range(len(RANKS)):
            m[f"w2_{i}"] = w2_parts[i]
        in_maps.append(m)
    return in_maps


def _install_ntff_hook():
    """Provide antenv.axon_hooks if the image lacks it (trace support).

    run_bass_kernel_spmd's axon trace path imports
    antenv.axon_hooks.get_axon_ntff_profile_hook; this container's antenv
    has no such module, so recreate the ctypes-based hook against the
    injected libaxon_pjrt.so (same as trn_agent_boot._ntff_profile_via_ctypes).
    """
    try:
        from antenv.axon_hooks import get_axon_ntff_profile_hook  # noqa: F401
        return
    except ImportError:
        pass
    so_path = "/opt/axon/libaxon_pjrt.so"
    hook = None
    if os.path.exists(so_path):
        lib = ctypes.CDLL(so_path)
        if hasattr(lib, "axon_start_nrt_profile"):
            lib.axon_start_nrt_profile.argtypes = [
                ctypes.POINTER(ctypes.c_int64),
                ctypes.c_size_t,
            ]
            lib.axon_start_nrt_profile.restype = ctypes.c_int64
            lib.axon_stop_nrt_profile.argtypes = [ctypes.c_char_p]
            lib.axon_stop_nrt_profile.restype = ctypes.c_int64

            @contextlib.contextmanager
            def _hook(output_dir, device_ids):
                import jax

                jax.devices()
                if device_ids:
                    ids = (ctypes.c_int64 * len(device_ids))(*device_ids)
                    rc = lib.axon_start_nrt_profile(ids, len(device_ids))
                else:
                    rc = lib.axon_start_nrt_profile(None, 0)
                if rc != 0:
                    raise RuntimeError(f"axon_start_nrt_profile rc={rc}")
                try:
                    yield
                finally:
                    n = lib.axon_stop_nrt_profile(str(output_dir).encode())
                    if n < 0:
                        raise RuntimeError(f"axon_stop_nrt_profile rc={n}")

            hook = _hook

    import antenv

    mod = types.ModuleType("antenv.axon_hooks")
    state = {"hook": hook}
    mod.get_axon_ntff_profile_hook = lambda: state["hook"]
    mod.set_axon_ntff_profile_hook = lambda h: state.__setitem__("hook", h)
    sys.modules["antenv.axon_hooks"] = mod
    antenv.axon_hooks = mod


def run(inputs, trace=False, tmpdir=None):
    """Run the kernel on all 8 cores; returns (full_output, BassKernelResults)."""
    with_b2 = any(
        np.any(np.asarray(inputs[f"b2_{i}"])) for i in range(len(RANKS))
    )
    if trace:
        _install_ntff_hook()
    nc = _get_program(with_b2)
    in_maps = _prepare_inputs(inputs, with_b2)
    res = run_bass_kernel_spmd(
        nc, in_maps, core_ids=list(range(NCORES)), trace=trace, tmpdir=tmpdir
    )
    out = np.concatenate(
        [res.results[i]["out"] for i in range(NCORES)], axis=0
    ).astype(np.float32)
    return out, res


def kernel(**inputs) -> np.ndarray:
    out, _ = run(inputs, trace=False)
    return out
